# revision 5
# baseline (speedup 1.0000x reference)
"""Multi-head attention (B=2, T=2048, C=1024, H=16, D=64) on 8 TRN2 cores.

Tensor-parallel over heads: each core owns 2 heads (128 channels).

v4: fully software-pipelined single phase. All matmul operands bf16
(fp32 matmuls cost 4 cycles/row on the PE, bf16 costs 1). The x/qkv
projections are split into per-chunk units (u_qk = q+k channel-major,
u_v = v token-major halves) and interleaved into the attention block
stream so ScalarE exp starts ~6us in and the PE never sits behind a
phase barrier. o_proj partials are queued and emitted inside the
second-batch blocks (their PSUM tile cycles the same bank pair as the
projection accumulators). Output stored bf16; host sums the 8 partials
+ bias in fp32.

Per core:
  - q,k projected channel-major (qT/kT [128, N] bf16); bk dropped
    (softmax shift-invariance), bq added at PSUM evict.
  - v projected token-major; bv folded into host-side bias.
  - attention with scoresT = k @ q.T layout ([ktok, qtok]); exp on
    ScalarE with the 1/sqrt(D) scale folded in; no max-subtraction.
  - v augmented with a ones column (lhsT M=65) so the PV matmul also
    accumulates softmax denominators in PSUM row 64.
  - normalize at PV evict: DVE reciprocal -> GpSimd partition-broadcast
    -> DVE mul.
"""

import numpy as np
import ml_dtypes

import concourse.bacc as bacc
import concourse.tile as tile
from concourse import mybir
from concourse.bass_utils import run_bass_kernel_spmd

NCORES = 8
B, T, C, H, D = 2, 2048, 1024, 16, 64
N = B * T  # 4096 tokens
CPC = 128  # channels per core (2 heads x 64)
SCALE = 0.125  # 1/sqrt(64)
F32 = mybir.dt.float32
BF16 = mybir.dt.bfloat16
NPBF16 = ml_dtypes.bfloat16

KT = C // 128  # 8 contraction tiles for projections
NCH = N // 512  # 8 token chunks for projections
TTOK = N // 128  # 32 token tiles
KTA = T // 128  # 16 k tiles per batch in attention
QC = T // 512  # 4 q chunks per batch
VS = 2 * (D + 1)  # 130: per-k-tile stride in v_aug (65 cols per head)

_CACHE = {}


def _build(dbg=False, reps=1, phases="ABC"):
    nc = bacc.Bacc("TRN2", target_bir_lowering=False, debug=False)

    xT = nc.dram_tensor("xT", [C, N], BF16, kind="ExternalInput")
    wqT = nc.dram_tensor("wqT", [C, CPC], BF16, kind="ExternalInput")
    wkT = nc.dram_tensor("wkT", [C, CPC], BF16, kind="ExternalInput")
    wvT = nc.dram_tensor("wvT", [C, CPC], BF16, kind="ExternalInput")
    woT = nc.dram_tensor("woT", [CPC, C], BF16, kind="ExternalInput")
    bq = nc.dram_tensor("bq", [CPC, 1], F32, kind="ExternalInput")
    out = nc.dram_tensor("out", [N, C], BF16, kind="ExternalOutput")
    if dbg:
        d_qT = nc.dram_tensor("d_qT", [128, N], BF16, kind="ExternalOutput")
        d_kT = nc.dram_tensor("d_kT", [128, N], BF16, kind="ExternalOutput")
        d_va = nc.dram_tensor("d_va", [128, TTOK * VS], BF16, kind="ExternalOutput")
        d_ao = nc.dram_tensor("d_ao", [128, N], BF16, kind="ExternalOutput")

    with tile.TileContext(nc) as tc, tc.tile_pool(name="persist", bufs=1) as persist:
        qT_sb = persist.tile([128, N], BF16, tag="qT")
        kT_sb = persist.tile([128, N], BF16, tag="kT")
        vaug = persist.tile([128, TTOK * VS], BF16, tag="vaug")
        aout = persist.tile([128, N], BF16, tag="aout")
        wq_sb = persist.tile([128, C], BF16, tag="wq")
        wk_sb = persist.tile([128, C], BF16, tag="wk")
        wv_sb = persist.tile([128, C], BF16, tag="wv")
        wo_sb = persist.tile([128, C], BF16, tag="wo")
        bq_sb = persist.tile([128, 1], F32, tag="bq")
        # enough of vaug for the HAM warmup operands; rest is set after
        # warmup is underway (disjoint columns, no dep)
        nc.vector.memset(vaug[:, 0:640], 1.0)

        for kt in range(KT):
            ksl = slice(kt * 128, (kt + 1) * 128)
            nc.sync.dma_start(out=wq_sb[:, ksl], in_=wqT[ksl, :])
            nc.sync.dma_start(out=wk_sb[:, ksl], in_=wkT[ksl, :])
            nc.sync.dma_start(out=wv_sb[:, ksl], in_=wvT[ksl, :])
        nc.sync.dma_start(out=wo_sb[:, :], in_=woT[:, :])
        nc.sync.dma_start(out=bq_sb[:, :], in_=bq[:, :])

        with (
            tc.tile_pool(name="xk", bufs=4) as xkp,
            tc.tile_pool(name="paP", bufs=2, space="PSUM") as paP,
            tc.tile_pool(name="psS", bufs=2, space="PSUM") as psS,
            tc.tile_pool(name="psP", bufs=1, space="PSUM") as psP,
            tc.tile_pool(name="aup", bufs=6) as aup,
            tc.tile_pool(name="nrm", bufs=4) as nrm,
            tc.tile_pool(name="ob", bufs=4) as obp,
        ):
          for rep in range(reps):
            if rep == 0:
                xk_tiles = {}  # chunk -> [128, 4096] bf16 tile (kt-major columns)

            def emit_dma(n, eng2=None):
                # 8 DMAs per chunk (one per kt tile) so the hw queues run in
                # parallel and consumers wait only on their covering piece.
                # eng2: optional second issuing engine to halve issue latency.
                cols = slice(n * 512, (n + 1) * 512)
                xc = xkp.tile([128, KT * 512], BF16, tag="xc", name=f"xc{n}")
                for kt in range(KT):
                    eng = nc.sync if (eng2 is None or kt % 2 == 0) else eng2
                    eng.dma_start(
                        out=xc[:, kt * 512 : (kt + 1) * 512],
                        in_=xT[kt * 128 : (kt + 1) * 128, cols],
                    )
                xk_tiles[n] = xc

            def emit_uq(n):
                # q projection for chunk n, channel-major
                cols = slice(n * 512, (n + 1) * 512)
                xc = xk_tiles[n]
                pa = paP.tile([128, 512], F32, tag="pa", name=f"uq{n}")
                for kt in range(KT):
                    nc.tensor.matmul(
                        pa[:, :],
                        lhsT=wq_sb[:, kt * 128 : (kt + 1) * 128],
                        rhs=xc[:, kt * 512 : (kt + 1) * 512],
                        start=kt == 0, stop=kt == KT - 1,
                    )
                nc.vector.tensor_scalar_add(
                    out=qT_sb[:, cols], in0=pa[:, :], scalar1=bq_sb[:, :]
                )

            def emit_uk(n):
                # k projection for chunk n, channel-major
                cols = slice(n * 512, (n + 1) * 512)
                xc = xk_tiles[n]
                pa = paP.tile([128, 512], F32, tag="pa", name=f"uk{n}")
                for kt in range(KT):
                    nc.tensor.matmul(
                        pa[:, :],
                        lhsT=wk_sb[:, kt * 128 : (kt + 1) * 128],
                        rhs=xc[:, kt * 512 : (kt + 1) * 512],
                        start=kt == 0, stop=kt == KT - 1,
                    )
                nc.vector.tensor_copy(out=kT_sb[:, cols], in_=pa[:, :])

            def emit_uv(n, half):
                # v projection for chunk n, token subtiles 2*half, 2*half+1
                xc = xk_tiles[n]
                pa = paP.tile([128, 512], F32, tag="pa", name=f"uv{n}_{half}")
                for j in range(2):
                    tt = 2 * half + j
                    for kt in range(KT):
                        ksl = slice(kt * 128, (kt + 1) * 128)
                        nc.tensor.matmul(
                            pa[:, j * 128 : (j + 1) * 128],
                            lhsT=xc[:, kt * 512 + tt * 128 : kt * 512 + (tt + 1) * 128],
                            rhs=wv_sb[:, ksl],
                            start=kt == 0, stop=kt == KT - 1,
                        )
                for j in range(2):
                    g = n * 4 + 2 * half + j  # global token tile
                    for h in range(2):
                        nc.vector.tensor_copy(
                            out=vaug[:, g * VS + h * 65 : g * VS + h * 65 + 64],
                            in_=pa[:, j * 128 + h * 64 : j * 128 + h * 64 + 64],
                        )

            def emit_evict(pv, qsl):
                # normalize: recip rows (DVE) -> GpSimd partition-broadcast
                # (Pool engine, otherwise idle; keeps PE and DVE free) ->
                # DVE mul straight from PSUM
                for h in range(2):
                    rc = nrm.tile([1, 512], BF16, tag="rc", name="rc")
                    with nc.allow_low_precision(reason="softmax denom recip"):
                        nc.vector.reciprocal(out=rc[:, :], in_=pv[h][64:65, :])
                    rc64 = nrm.tile([64, 512], BF16, tag="rc64", name="rc64")
                    nc.gpsimd.partition_broadcast(rc64[:, :], rc[:, :], channels=64)
                    nc.vector.tensor_mul(
                        out=aout[h * 64 : (h + 1) * 64, qsl],
                        in0=pv[h][0:64, :],
                        in1=rc64[:, :],
                    )

            if rep == 0:
                po_pair = []  # (t0, ob_tile) of the pending even po

            def emit_po(t0):
                # o_proj partial for token tile starting at t0; paired output
                # DMA: two consecutive token tiles share one [128, 2048] ob
                # tile and a single merged DMA
                if not po_pair:
                    ob = obp.tile([128, 2 * 1024], BF16, tag="ob")
                    po_pair.append((t0, ob))
                    off = 0
                else:
                    ob = po_pair[0][1]
                    off = 1024
                for nh in range(2):
                    po = paP.tile([128, 512], F32, tag="pa", name="po")
                    nc.tensor.matmul(
                        po[:, :],
                        lhsT=aout[:, t0 : t0 + 128],
                        rhs=wo_sb[:, nh * 512 : (nh + 1) * 512],
                        start=True, stop=True,
                    )
                    nc.vector.tensor_copy(
                        out=ob[:, off + nh * 512 : off + (nh + 1) * 512], in_=po[:, :]
                    )
                if off == 1024:
                    t0e = po_pair.pop(0)[0]
                    assert t0 == t0e + 128
                    # issue from Pool: SP's queue stays free for x prefetch,
                    # and this descriptor is expensive to build (~1.6us)
                    nc.gpsimd.dma_start(
                        out=out[t0e : t0e + 256, :].rearrange("(j p) c -> p j c", j=2),
                        in_=ob[:, :].rearrange("p (j c) -> p j c", j=2),
                    )

            # filler schedule: per block index, {iter: [callable, ...]}
            def F(fn, *a):
                return lambda: fn(*a)

            fillers = {
                0: {0: [F(emit_dma, 3), F(emit_uk, 1)], 1: [F(emit_uv, 1, 0)],
                    2: [F(emit_uk, 2)], 3: [F(emit_uv, 1, 1)],
                    4: [F(emit_uk, 3)], 5: [F(emit_uv, 2, 0)],
                    6: [F(emit_uv, 2, 1)], 8: [F(emit_uv, 3, 0)],
                    10: [F(emit_uv, 3, 1)], 12: [F(emit_uq, 1)]},
                1: {0: [F(emit_dma, 4), F(emit_uq, 2)], 2: [F(emit_uk, 4)],
                    4: [F(emit_uv, 4, 0)], 6: [F(emit_uv, 4, 1)]},
                2: {0: [F(emit_dma, 5), F(emit_uq, 3)], 2: [F(emit_uk, 5)],
                    4: [F(emit_uv, 5, 0)], 6: [F(emit_uv, 5, 1)]},
                3: {0: [F(emit_dma, 6), F(emit_uq, 4)], 2: [F(emit_uk, 6)],
                    4: [F(emit_uv, 6, 0)], 6: [F(emit_uv, 6, 1)],
                    8: [F(emit_dma, 7)]},
                4: {0: [F(emit_uq, 5)], 1: [F(emit_uk, 7)],
                    2: [F(emit_uv, 7, 0)], 3: [F(emit_uv, 7, 1)]},
                5: {0: [F(emit_uq, 6)]},
                6: {0: [F(emit_uq, 7)]},
            }
            if rep + 1 < reps:
                # prefetch next rep's chunk 0-2 into the tail blocks (batch-0
                # SBUF regions are dead once the b1 blocks start)
                fillers[5][2] = [F(emit_dma, 0)]
                fillers[5][4] = [F(emit_uk, 0)]
                fillers[6][2] = [F(emit_dma, 1), F(emit_uq, 0)]
                fillers[6][4] = [F(emit_uv, 0, 0)]
                fillers[7] = {1: [F(emit_uv, 0, 1)], 3: [F(emit_dma, 2)]}
            # po slots: block -> iters where one queued o_proj tile is emitted
            # (32 slots/rep: 28 own evict tiles + 4 carried from the previous
            # rep's last block, whose evict defers into this rep's block 0)
            po_slots = {
                4: (5, 7, 9, 11, 13, 15),
                5: (0, 2, 3, 5, 7, 9, 11, 13, 15),
                6: (0, 2, 3, 5, 7, 9, 11, 13, 15),
                7: (0, 2, 4, 6, 8, 9, 10, 12),
            }
            if rep == 0:
                po_queue = []

            # ---- pre-section ------------------------------------------
            if rep == 0:
                # HAM warmup on the vaug memset; bridges the first DMAs.
                wup = paP.tile([128, 512], F32, tag="pa", name="wup")
                for _ in range(28):
                    nc.tensor.matmul(
                        wup[:, :], lhsT=vaug[:, 0:128], rhs=vaug[:, 0:512],
                        start=True, stop=True,
                    )
                nc.vector.memset(vaug[:, 640:], 1.0)
                emit_dma(0, eng2=nc.scalar)
                emit_dma(1, eng2=nc.scalar)
                emit_dma(2, eng2=nc.scalar)
                emit_uk(0)
                emit_uq(0)
                emit_uv(0, 0)
                emit_uv(0, 1)

            # ---- block stream -----------------------------------------
            if rep == 0:
                prev = None
            for bi in range(B * QC):
                b, qc = divmod(bi, QC)
                q0 = b * T + qc * 512
                qsl = slice(q0, q0 + 512)
                pv = [
                    psP.tile([128, 512], F32, tag=f"pv{h}", name=f"pv{h}")
                    for h in range(2)
                ]
                sc_t = [None] * KTA
                au_t = [None] * KTA

                def emit_qk(kt, b=b, qsl=qsl, sc_t=sc_t):
                    sc = psS.tile([128, 1024], F32, tag="sc", name="sc")
                    sc_t[kt] = sc
                    kcols = slice(b * T + kt * 128, b * T + (kt + 1) * 128)
                    for h in range(2):
                        hp = slice(h * 64, (h + 1) * 64)
                        nc.tensor.matmul(
                            sc[:, h * 512 : (h + 1) * 512],
                            lhsT=kT_sb[hp, kcols],
                            rhs=qT_sb[hp, qsl],
                            start=True, stop=True,
                        )

                def emit_exp(kt, sc_t=sc_t, au_t=au_t):
                    au = aup.tile([128, 1024], BF16, tag="au", name="au")
                    au_t[kt] = au
                    nc.scalar.activation(
                        out=au[:, :],
                        in_=sc_t[kt][:, :],
                        func=mybir.ActivationFunctionType.Exp,
                        scale=SCALE,
                    )
                    sc_t[kt] = None

                def emit_pv(kt, b=b, pv=pv, au_t=au_t):
                    g = b * KTA + kt
                    for h in range(2):
                        nc.tensor.matmul(
                            pv[h][0:65, :],
                            lhsT=vaug[:, g * VS + h * 65 : g * VS + (h + 1) * 65],
                            rhs=au_t[kt][:, h * 512 : (h + 1) * 512],
                            start=(kt == 0), stop=(kt == KTA - 1),
                        )
                    au_t[kt] = None

                blk_fill = fillers.get(bi, {})
                blk_po = po_slots.get(bi, ())
                emit_qk(0)
                if prev is not None:
                    emit_evict(prev[0], prev[1])
                    for tt in range(4):
                        po_queue.append(prev[2] + tt * 128)
                emit_qk(1)
                for kt in range(KTA):
                    emit_exp(kt)
                    for fn in blk_fill.get(kt, ()):
                        fn()
                    if kt in blk_po and po_queue:
                        emit_po(po_queue.pop(0))
                    if kt + 2 < KTA:
                        emit_qk(kt + 2)
                    emit_pv(kt)
                prev = (pv, qsl, q0)

            # ---- tail (last rep only; other reps carry prev/queue over
            # so the next rep's early blocks absorb the drain) ----------
            if rep == reps - 1:
                emit_evict(prev[0], prev[1])
                for tt in range(4):
                    po_queue.append(prev[2] + tt * 128)
                while po_queue:
                    emit_po(po_queue.pop(0))

            if dbg:
                nc.sync.dma_start(out=d_qT[:, :], in_=qT_sb[:, :])
                nc.sync.dma_start(out=d_kT[:, :], in_=kT_sb[:, :])
                nc.sync.dma_start(out=d_va[:, :], in_=vaug[:, :])
                nc.sync.dma_start(out=d_ao[:, :], in_=aout[:, :])

    nc.compile()
    return nc


def _prep_inputs(x_q, Wq, bq, Wk, Wv, Wo):
    x = np.ascontiguousarray(np.asarray(x_q, np.float32).reshape(N, C))
    xT = np.ascontiguousarray(x.T.astype(NPBF16))
    Wq = np.asarray(Wq, np.float32)
    Wk = np.asarray(Wk, np.float32)
    Wv = np.asarray(Wv, np.float32)
    Wo = np.asarray(Wo, np.float32)
    bq = np.asarray(bq, np.float32)
    in_maps = []
    for c in range(NCORES):
        sl = slice(c * CPC, (c + 1) * CPC)
        in_maps.append(
            {
                "xT": xT,
                "wqT": np.ascontiguousarray(Wq[sl, :].T.astype(NPBF16)),
                "wkT": np.ascontiguousarray(Wk[sl, :].T.astype(NPBF16)),
                "wvT": np.ascontiguousarray(Wv[sl, :].T.astype(NPBF16)),
                "woT": np.ascontiguousarray(Wo[:, sl].T.astype(NPBF16)),
                "bq": np.ascontiguousarray(bq[sl].reshape(CPC, 1)),
            }
        )
    return in_maps


def _finish(results, Wo, bv, bo):
    acc = results[0]["out"].astype(np.float32)
    for r in results[1:]:
        acc = acc + r["out"].astype(np.float32)
    bo_eff = np.asarray(bo, np.float32) + np.asarray(Wo, np.float32) @ np.asarray(
        bv, np.float32
    )
    return (acc + bo_eff[None, :]).reshape(B, T, C).astype(np.float32)


def run(inputs, trace=False, **kw):
    if "nc" not in _CACHE:
        _CACHE["nc"] = _build()
    nc = _CACHE["nc"]
    in_maps = _prep_inputs(
        inputs["x_q"], inputs["Wq"], inputs["bq"], inputs["Wk"], inputs["Wv"],
        inputs["Wo"],
    )
    res = run_bass_kernel_spmd(nc, in_maps, core_ids=list(range(NCORES)),
                               trace=trace, **kw)
    out = _finish(res.results, inputs["Wo"], inputs["bv"], inputs["bo"])
    return out, res


def kernel(**inputs):
    out, _ = run(inputs)
    return out



# revision 6
# speedup vs baseline: 1.0772x; 1.0772x over previous
"""Multi-head attention (B=2, T=2048, C=1024, H=16, D=64) on 8 TRN2 cores.

Tensor-parallel over heads: each core owns 2 heads (128 channels).

v4: fully software-pipelined single phase. All matmul operands bf16
(fp32 matmuls cost 4 cycles/row on the PE, bf16 costs 1). The x/qkv
projections are split into per-chunk units (u_qk = q+k channel-major,
u_v = v token-major halves) and interleaved into the attention block
stream so ScalarE exp starts ~6us in and the PE never sits behind a
phase barrier. o_proj partials are queued and emitted inside the
second-batch blocks (their PSUM tile cycles the same bank pair as the
projection accumulators). Output stored bf16; host sums the 8 partials
+ bias in fp32.

Per core:
  - q,k projected channel-major (qT/kT [128, N] bf16); bk dropped
    (softmax shift-invariance), bq added at PSUM evict.
  - v projected token-major; bv folded into host-side bias.
  - attention with scoresT = k @ q.T layout ([ktok, qtok]); exp on
    ScalarE with the 1/sqrt(D) scale folded in; no max-subtraction.
  - v augmented with a ones column (lhsT M=65) so the PV matmul also
    accumulates softmax denominators in PSUM row 64.
  - normalize at PV evict: DVE reciprocal -> GpSimd partition-broadcast
    -> DVE mul.
"""

import numpy as np
import ml_dtypes

import concourse.bacc as bacc
import concourse.tile as tile
from concourse import mybir
from concourse.bass_utils import run_bass_kernel_spmd

NCORES = 8
B, T, C, H, D = 2, 2048, 1024, 16, 64
N = B * T  # 4096 tokens
CPC = 128  # channels per core (2 heads x 64)
SCALE = 0.125  # 1/sqrt(64)
F32 = mybir.dt.float32
BF16 = mybir.dt.bfloat16
NPBF16 = ml_dtypes.bfloat16

KT = C // 128  # 8 contraction tiles for projections
NCH = N // 512  # 8 token chunks for projections
TTOK = N // 128  # 32 token tiles
KTA = T // 128  # 16 k tiles per batch in attention
QC = T // 512  # 4 q chunks per batch
VS = 2 * (D + 1)  # 130: per-k-tile stride in v_aug (65 cols per head)

_CACHE = {}


def _build(dbg=False, reps=1, phases="ABC"):
    nc = bacc.Bacc("TRN2", target_bir_lowering=False, debug=False)

    xT = nc.dram_tensor("xT", [C, N], BF16, kind="ExternalInput")
    wqT = nc.dram_tensor("wqT", [C, CPC], BF16, kind="ExternalInput")
    wkT = nc.dram_tensor("wkT", [C, CPC], BF16, kind="ExternalInput")
    wvT = nc.dram_tensor("wvT", [C, CPC], BF16, kind="ExternalInput")
    woT = nc.dram_tensor("woT", [CPC, C], BF16, kind="ExternalInput")
    bq = nc.dram_tensor("bq", [CPC, 1], F32, kind="ExternalInput")
    out = nc.dram_tensor("out", [N, C], BF16, kind="ExternalOutput")
    if dbg:
        d_qT = nc.dram_tensor("d_qT", [128, N], BF16, kind="ExternalOutput")
        d_kT = nc.dram_tensor("d_kT", [128, N], BF16, kind="ExternalOutput")
        d_va = nc.dram_tensor("d_va", [128, TTOK * VS], BF16, kind="ExternalOutput")
        d_ao = nc.dram_tensor("d_ao", [128, N], BF16, kind="ExternalOutput")

    with tile.TileContext(nc) as tc, tc.tile_pool(name="persist", bufs=1) as persist:
        qT_sb = persist.tile([128, N], BF16, tag="qT")
        kT_sb = persist.tile([128, N], BF16, tag="kT")
        vaug = persist.tile([128, TTOK * VS], BF16, tag="vaug")
        aout = persist.tile([128, N], BF16, tag="aout")
        wq_sb = persist.tile([128, C], BF16, tag="wq")
        wk_sb = persist.tile([128, C], BF16, tag="wk")
        wv_sb = persist.tile([128, C], BF16, tag="wv")
        wo_sb = persist.tile([128, C], BF16, tag="wo")
        bq_sb = persist.tile([128, 1], F32, tag="bq")
        # enough of vaug for the HAM warmup operands; rest is set after
        # warmup is underway (disjoint columns, no dep)
        nc.vector.memset(vaug[:, 0:640], 1.0)

        for kt in range(KT):
            ksl = slice(kt * 128, (kt + 1) * 128)
            nc.sync.dma_start(out=wq_sb[:, ksl], in_=wqT[ksl, :])
            nc.sync.dma_start(out=wk_sb[:, ksl], in_=wkT[ksl, :])
            nc.sync.dma_start(out=wv_sb[:, ksl], in_=wvT[ksl, :])
        nc.sync.dma_start(out=wo_sb[:, :], in_=woT[:, :])
        nc.sync.dma_start(out=bq_sb[:, :], in_=bq[:, :])

        with (
            tc.tile_pool(name="xk", bufs=4) as xkp,
            tc.tile_pool(name="paP", bufs=2, space="PSUM") as paP,
            tc.tile_pool(name="psS", bufs=2, space="PSUM") as psS,
            tc.tile_pool(name="psP", bufs=1, space="PSUM") as psP,
            tc.tile_pool(name="aup", bufs=6) as aup,
            tc.tile_pool(name="nrm", bufs=4) as nrm,
            tc.tile_pool(name="ob", bufs=4) as obp,
        ):
          for rep in range(reps):
            if rep == 0:
                xk_tiles = {}  # chunk -> [128, 4096] bf16 tile (kt-major columns)

            def emit_dma(n, eng2=None):
                # 8 DMAs per chunk (one per kt tile) so the hw queues run in
                # parallel and consumers wait only on their covering piece.
                # eng2: optional second issuing engine to halve issue latency.
                cols = slice(n * 512, (n + 1) * 512)
                xc = xkp.tile([128, KT * 512], BF16, tag="xc", name=f"xc{n}")
                for kt in range(KT):
                    eng = nc.sync if (eng2 is None or kt % 2 == 0) else eng2
                    eng.dma_start(
                        out=xc[:, kt * 512 : (kt + 1) * 512],
                        in_=xT[kt * 128 : (kt + 1) * 128, cols],
                    )
                xk_tiles[n] = xc

            def emit_uq(n):
                # q projection for chunk n, channel-major
                cols = slice(n * 512, (n + 1) * 512)
                xc = xk_tiles[n]
                pa = paP.tile([128, 512], F32, tag="pa", name=f"uq{n}")
                for kt in range(KT):
                    nc.tensor.matmul(
                        pa[:, :],
                        lhsT=wq_sb[:, kt * 128 : (kt + 1) * 128],
                        rhs=xc[:, kt * 512 : (kt + 1) * 512],
                        start=kt == 0, stop=kt == KT - 1,
                    )
                nc.vector.tensor_scalar_add(
                    out=qT_sb[:, cols], in0=pa[:, :], scalar1=bq_sb[:, :]
                )

            def emit_uk(n):
                # k projection for chunk n, channel-major
                cols = slice(n * 512, (n + 1) * 512)
                xc = xk_tiles[n]
                pa = paP.tile([128, 512], F32, tag="pa", name=f"uk{n}")
                for kt in range(KT):
                    nc.tensor.matmul(
                        pa[:, :],
                        lhsT=wk_sb[:, kt * 128 : (kt + 1) * 128],
                        rhs=xc[:, kt * 512 : (kt + 1) * 512],
                        start=kt == 0, stop=kt == KT - 1,
                    )
                nc.vector.tensor_copy(out=kT_sb[:, cols], in_=pa[:, :])

            def emit_uv(n, half):
                # v projection for chunk n, token subtiles 2*half, 2*half+1
                xc = xk_tiles[n]
                pa = paP.tile([128, 512], F32, tag="pa", name=f"uv{n}_{half}")
                for j in range(2):
                    tt = 2 * half + j
                    for kt in range(KT):
                        ksl = slice(kt * 128, (kt + 1) * 128)
                        nc.tensor.matmul(
                            pa[:, j * 128 : (j + 1) * 128],
                            lhsT=xc[:, kt * 512 + tt * 128 : kt * 512 + (tt + 1) * 128],
                            rhs=wv_sb[:, ksl],
                            start=kt == 0, stop=kt == KT - 1,
                        )
                for j in range(2):
                    g = n * 4 + 2 * half + j  # global token tile
                    for h in range(2):
                        nc.vector.tensor_copy(
                            out=vaug[:, g * VS + h * 65 : g * VS + h * 65 + 64],
                            in_=pa[:, j * 128 + h * 64 : j * 128 + h * 64 + 64],
                        )

            def emit_evict(pv, qsl):
                # normalize: recip rows (DVE) -> GpSimd partition-broadcast
                # (Pool engine, otherwise idle; keeps PE and DVE free) ->
                # DVE mul straight from PSUM
                for h in range(2):
                    rc = nrm.tile([1, 512], BF16, tag="rc", name="rc")
                    with nc.allow_low_precision(reason="softmax denom recip"):
                        nc.vector.reciprocal(out=rc[:, :], in_=pv[h][64:65, :])
                    rc64 = nrm.tile([64, 512], BF16, tag="rc64", name="rc64")
                    nc.gpsimd.partition_broadcast(rc64[:, :], rc[:, :], channels=64)
                    nc.vector.tensor_mul(
                        out=aout[h * 64 : (h + 1) * 64, qsl],
                        in0=pv[h][0:64, :],
                        in1=rc64[:, :],
                    )

            if rep == 0:
                po_pair = []  # (t0, ob_tile) of the pending even po

            def emit_po(t0):
                # o_proj partial for token tile starting at t0; paired output
                # DMA: two consecutive token tiles share one [128, 2048] ob
                # tile and a single merged DMA
                if not po_pair:
                    ob = obp.tile([128, 2 * 1024], BF16, tag="ob")
                    po_pair.append((t0, ob))
                    off = 0
                else:
                    ob = po_pair[0][1]
                    off = 1024
                for nh in range(2):
                    po = paP.tile([128, 512], F32, tag="pa", name="po")
                    nc.tensor.matmul(
                        po[:, :],
                        lhsT=aout[:, t0 : t0 + 128],
                        rhs=wo_sb[:, nh * 512 : (nh + 1) * 512],
                        start=True, stop=True,
                    )
                    nc.vector.tensor_copy(
                        out=ob[:, off + nh * 512 : off + (nh + 1) * 512], in_=po[:, :]
                    )
                if off == 1024:
                    t0e = po_pair.pop(0)[0]
                    assert t0 == t0e + 128
                    nc.sync.dma_start(
                        out=out[t0e : t0e + 256, :].rearrange("(j p) c -> p j c", j=2),
                        in_=ob[:, :].rearrange("p (j c) -> p j c", j=2),
                    )

            # filler schedule: per block index, {iter: [callable, ...]}
            def F(fn, *a):
                return lambda: fn(*a)

            fillers = {
                0: {0: [F(emit_dma, 3), F(emit_uk, 1)], 1: [F(emit_uv, 1, 0)],
                    2: [F(emit_uk, 2)], 3: [F(emit_uv, 1, 1)],
                    4: [F(emit_uk, 3)], 5: [F(emit_uv, 2, 0)],
                    6: [F(emit_uv, 2, 1)], 8: [F(emit_uv, 3, 0)],
                    10: [F(emit_uv, 3, 1)], 12: [F(emit_uq, 1)]},
                1: {0: [F(emit_dma, 4), F(emit_uq, 2)], 2: [F(emit_uk, 4)],
                    4: [F(emit_uv, 4, 0)], 6: [F(emit_uv, 4, 1)]},
                2: {0: [F(emit_dma, 5), F(emit_uq, 3)], 2: [F(emit_uk, 5)],
                    4: [F(emit_uv, 5, 0)], 6: [F(emit_uv, 5, 1)]},
                3: {0: [F(emit_dma, 6), F(emit_uq, 4)], 2: [F(emit_uk, 6)],
                    4: [F(emit_uv, 6, 0)], 6: [F(emit_uv, 6, 1)],
                    8: [F(emit_dma, 7)]},
                4: {0: [F(emit_uq, 5)], 1: [F(emit_uk, 7)],
                    2: [F(emit_uv, 7, 0)], 3: [F(emit_uv, 7, 1)]},
                5: {0: [F(emit_uq, 6)]},
                6: {0: [F(emit_uq, 7)]},
            }
            if rep + 1 < reps:
                # prefetch next rep's chunk 0-2 into the tail blocks (batch-0
                # SBUF regions are dead once the b1 blocks start)
                fillers[5][2] = [F(emit_dma, 0)]
                fillers[5][4] = [F(emit_uk, 0)]
                fillers[6][2] = [F(emit_dma, 1), F(emit_uq, 0)]
                fillers[6][4] = [F(emit_uv, 0, 0)]
                fillers[7] = {1: [F(emit_uv, 0, 1)], 3: [F(emit_dma, 2)]}
            # po slots: block -> iters where one queued o_proj tile is emitted
            # (32 slots/rep: 28 own evict tiles + 4 carried from the previous
            # rep's last block, whose evict defers into this rep's block 0)
            po_slots = {
                4: (5, 7, 9, 11, 13, 15),
                5: (0, 2, 3, 5, 7, 9, 11, 13, 15),
                6: (0, 2, 3, 5, 7, 9, 11, 13, 15),
                7: (0, 2, 4, 6, 8, 9, 10, 12),
            }
            if rep == 0:
                po_queue = []

            # ---- pre-section ------------------------------------------
            if rep == 0:
                # HAM warmup on the vaug memset; bridges the first DMAs.
                wup = paP.tile([128, 512], F32, tag="pa", name="wup")
                for _ in range(28):
                    nc.tensor.matmul(
                        wup[:, :], lhsT=vaug[:, 0:128], rhs=vaug[:, 0:512],
                        start=True, stop=True,
                    )
                nc.vector.memset(vaug[:, 640:], 1.0)
                emit_dma(0, eng2=nc.scalar)
                emit_dma(1, eng2=nc.scalar)
                emit_dma(2, eng2=nc.scalar)
                emit_uk(0)
                emit_uq(0)
                emit_uv(0, 0)
                emit_uv(0, 1)

            # ---- block stream -----------------------------------------
            if rep == 0:
                prev = None
            for bi in range(B * QC):
                b, qc = divmod(bi, QC)
                q0 = b * T + qc * 512
                qsl = slice(q0, q0 + 512)
                pv = [
                    psP.tile([128, 512], F32, tag=f"pv{h}", name=f"pv{h}")
                    for h in range(2)
                ]
                sc_t = [None] * KTA
                au_t = [None] * KTA

                def emit_qk(kt, b=b, qsl=qsl, sc_t=sc_t):
                    sc = psS.tile([128, 1024], F32, tag="sc", name="sc")
                    sc_t[kt] = sc
                    kcols = slice(b * T + kt * 128, b * T + (kt + 1) * 128)
                    for h in range(2):
                        hp = slice(h * 64, (h + 1) * 64)
                        nc.tensor.matmul(
                            sc[:, h * 512 : (h + 1) * 512],
                            lhsT=kT_sb[hp, kcols],
                            rhs=qT_sb[hp, qsl],
                            start=True, stop=True,
                        )

                def emit_exp(kt, sc_t=sc_t, au_t=au_t):
                    au = aup.tile([128, 1024], BF16, tag="au", name="au")
                    au_t[kt] = au
                    nc.scalar.activation(
                        out=au[:, :],
                        in_=sc_t[kt][:, :],
                        func=mybir.ActivationFunctionType.Exp,
                        scale=SCALE,
                    )
                    sc_t[kt] = None

                def emit_pv(kt, b=b, pv=pv, au_t=au_t):
                    g = b * KTA + kt
                    for h in range(2):
                        nc.tensor.matmul(
                            pv[h][0:65, :],
                            lhsT=vaug[:, g * VS + h * 65 : g * VS + (h + 1) * 65],
                            rhs=au_t[kt][:, h * 512 : (h + 1) * 512],
                            start=(kt == 0), stop=(kt == KTA - 1),
                        )
                    au_t[kt] = None

                blk_fill = fillers.get(bi, {})
                blk_po = po_slots.get(bi, ())
                emit_qk(0)
                if prev is not None:
                    emit_evict(prev[0], prev[1])
                    for tt in range(4):
                        po_queue.append(prev[2] + tt * 128)
                emit_qk(1)
                for kt in range(KTA):
                    emit_exp(kt)
                    for fn in blk_fill.get(kt, ()):
                        fn()
                    if kt in blk_po and po_queue:
                        emit_po(po_queue.pop(0))
                    if kt + 2 < KTA:
                        emit_qk(kt + 2)
                    emit_pv(kt)
                prev = (pv, qsl, q0)

            # ---- tail (last rep only; other reps carry prev/queue over
            # so the next rep's early blocks absorb the drain) ----------
            if rep == reps - 1:
                emit_evict(prev[0], prev[1])
                for tt in range(4):
                    po_queue.append(prev[2] + tt * 128)
                while po_queue:
                    emit_po(po_queue.pop(0))

            if dbg:
                nc.sync.dma_start(out=d_qT[:, :], in_=qT_sb[:, :])
                nc.sync.dma_start(out=d_kT[:, :], in_=kT_sb[:, :])
                nc.sync.dma_start(out=d_va[:, :], in_=vaug[:, :])
                nc.sync.dma_start(out=d_ao[:, :], in_=aout[:, :])

    nc.compile()
    return nc


def _prep_inputs(x_q, Wq, bq, Wk, Wv, Wo):
    x = np.ascontiguousarray(np.asarray(x_q, np.float32).reshape(N, C))
    xT = np.ascontiguousarray(x.T.astype(NPBF16))
    Wq = np.asarray(Wq, np.float32)
    Wk = np.asarray(Wk, np.float32)
    Wv = np.asarray(Wv, np.float32)
    Wo = np.asarray(Wo, np.float32)
    bq = np.asarray(bq, np.float32)
    in_maps = []
    for c in range(NCORES):
        sl = slice(c * CPC, (c + 1) * CPC)
        in_maps.append(
            {
                "xT": xT,
                "wqT": np.ascontiguousarray(Wq[sl, :].T.astype(NPBF16)),
                "wkT": np.ascontiguousarray(Wk[sl, :].T.astype(NPBF16)),
                "wvT": np.ascontiguousarray(Wv[sl, :].T.astype(NPBF16)),
                "woT": np.ascontiguousarray(Wo[:, sl].T.astype(NPBF16)),
                "bq": np.ascontiguousarray(bq[sl].reshape(CPC, 1)),
            }
        )
    return in_maps


def _finish(results, Wo, bv, bo):
    acc = results[0]["out"].astype(np.float32)
    for r in results[1:]:
        acc = acc + r["out"].astype(np.float32)
    bo_eff = np.asarray(bo, np.float32) + np.asarray(Wo, np.float32) @ np.asarray(
        bv, np.float32
    )
    return (acc + bo_eff[None, :]).reshape(B, T, C).astype(np.float32)


def run(inputs, trace=False, **kw):
    if "nc" not in _CACHE:
        _CACHE["nc"] = _build()
    nc = _CACHE["nc"]
    in_maps = _prep_inputs(
        inputs["x_q"], inputs["Wq"], inputs["bq"], inputs["Wk"], inputs["Wv"],
        inputs["Wo"],
    )
    res = run_bass_kernel_spmd(nc, in_maps, core_ids=list(range(NCORES)),
                               trace=trace, **kw)
    out = _finish(res.results, inputs["Wo"], inputs["bv"], inputs["bo"])
    return out, res


def kernel(**inputs):
    out, _ = run(inputs)
    return out

